# revision 43
# baseline (speedup 1.0000x reference)
"""Multi-head attention kernel for 8 Trainium2 NeuronCores.

Problem: nn_MultiHeadAttention_49246095016569
  q,k,v: [S=2048, B=2, E=512] f32; per-head projections Wq/Wk/Wv [64,64],
  output FC Wfc [512,512] + bfc [512].
  The reference reshapes [S,B,E] -> [B,H,S,D] with a PLAIN reshape, so each
  (b,h) pair is a contiguous [2048,64] chunk of the flattened input.  There
  are 16 chunks; each of the 8 cores handles 2 chunks, fully independently
  (no collectives).  Output rows [512*i, 512*(i+1)) of the flattened
  [4096,512] output come from core i.

Math per chunk c (qc,kc,vc = [2048,64] slices):
  khp = kc @ g_t            (g_t = Wk.T @ Wq folds both QK projections)
  S   = qc @ khp.T          (= Q @ K.T exactly, up to rounding)
  P   = exp(S/8)            (softmax without max-subtraction; |S/8| < ~6)
  A   = (P @ (vc @ Wv.T)) / P.sum(axis=1)
  out_rows = A.reshape(256,512) @ Wfc.T + bfc

On-chip layout: everything is computed transposed (S^T tiles = khpT.T @ qhT)
so that softmax sums come free via a ones-column prepended to V', and the FC
contraction can slice A^T directly.

Perf structure (v2):
  - Score matmuls are K=64: ROW-TILED pairs (tile T0 = SBUF partitions 0-63,
    T8 = partitions 64-127) run two kt-tiles CONCURRENTLY on the two halves
    of the PE array.  Requires qhT duplicated into partitions 64-127 and
    khpT/khT in a split-partition layout (even s-tiles low, odd high) which
    the PE transposes produce naturally.
  - Softmax sums ride the PV matmul as a ones-column (pav partition 64); the
    sums row copies to partition 0 for the approx reciprocal, casts to fp16
    (so the K=1 broadcast matmul streams at full fp16 rate), and the
    normalization multiply reads the broadcast result straight from PSUM.
  - No transpose-mode warmup filler (transposes don't register as HAM
    activity); a short burst of real matmuls at kernel start warms the clock
    while the first DMAs land, and the kernel body keeps the PE dense.
  - Chunk-0 prep evacuations that aren't on the critical khT->khp->scores
    chain run on the then-idle Scalar engine.
  (GPSIMD is deliberately unused: each op triggers a ~8us Q7 LIBRARY_RELOAD
  that stalls the in-order PE queue at the injection points.)
"""

import numpy as np

import concourse.bass as bass
import concourse.mybir as mybir
import concourse.tile as tile
from concourse import bacc
from concourse import bass_utils
from concourse.masks import make_identity

F32 = mybir.dt.float32
F16 = mybir.dt.float16

S = 2048
D = 64
E = 512
NCORES = 8
CHUNKS_PER_CORE = 2
KT = S // 128  # 16 k-tiles of 128
QB = S // 512  # 4 q-blocks of 512

MM_DT = F16
ACT_EXP = mybir.ActivationFunctionType.Exp


def build_core_program():
    nc = bacc.Bacc(trn_type="TRN2")

    q_in = nc.dram_tensor("q_in", (CHUNKS_PER_CORE * S, D), MM_DT, kind="ExternalInput")
    k_in = nc.dram_tensor("k_in", (CHUNKS_PER_CORE * S, D), MM_DT, kind="ExternalInput")
    v_in = nc.dram_tensor("v_in", (CHUNKS_PER_CORE * S, D), MM_DT, kind="ExternalInput")
    g_t = nc.dram_tensor("g_t", (D, D), MM_DT, kind="ExternalInput")
    wv_t = nc.dram_tensor("wv_t", (D, D), MM_DT, kind="ExternalInput")
    wfc_t = nc.dram_tensor("wfc_t", (E, E), MM_DT, kind="ExternalInput")
    bias = nc.dram_tensor("bias", (1, E), F32, kind="ExternalInput")
    out = nc.dram_tensor("out", (CHUNKS_PER_CORE * 256, E), F32, kind="ExternalOutput")

    with tile.TileContext(nc) as tc:
        with (
            tc.tile_pool(name="consts", bufs=1) as consts,
            tc.tile_pool(name="raw", bufs=2) as raw_pool,
            tc.tile_pool(name="tp", bufs=2) as tp_pool,
            tc.tile_pool(name="pt", bufs=6) as pt_pool,
            tc.tile_pool(name="at", bufs=2) as at_pool,
            tc.tile_pool(name="outp", bufs=2) as out_pool,
            tc.tile_pool(name="npool", bufs=2) as npool,
            tc.tile_pool(name="ps_small", bufs=2, space="PSUM") as ps_small,
            tc.tile_pool(name="ps_score", bufs=2, space="PSUM") as ps_score,
            tc.tile_pool(name="ps_acc", bufs=2, space="PSUM") as ps_acc,
        ):
            identity = consts.tile([128, 128], MM_DT)
            make_identity(nc, identity[:])

            # g2: folded QK projection, duplicated into both partition halves
            # so the odd-kt khp matmul can run as row-tile T8.  The const
            # DMAs are issued AFTER the chunk raw loads (emit_consts_dmas)
            # so the 512KB wfc load doesn't delay the critical k/q/v path.
            g2 = consts.tile([128, D], MM_DT)
            wv_sb = consts.tile([D, D], MM_DT)
            # Wfc.T as [64, 8, 512]: slice j = wfc_sb[:, j, :] (base partition 0)
            wfc_sb = consts.tile([D, 8, E], MM_DT)
            bias_sb = consts.tile([1, E], F32)

            def emit_consts_dmas():
                nc.sync.dma_start(g2[0:D, :], g_t[:])
                nc.sync.dma_start(g2[D:128, :], g_t[:])
                nc.sync.dma_start(wv_sb[:], wv_t[:])
                nc.sync.dma_start(
                    wfc_sb[:], wfc_t[:].rearrange("(j d) e -> d j e", d=D)
                )
                nc.sync.dma_start(bias_sb[:], bias[:])
            ones1 = consts.tile([1, 128], F32)
            nc.vector.memset(ones1[:], 1.0)
            ones_col = consts.tile([128, KT, 1], F32)
            nc.vector.memset(ones_col[:], 1.0)
            ones_row = consts.tile([1, D], MM_DT)
            nc.vector.memset(ones_row[:], 1.0)

            # ---- HAM warm-up: real (non-transpose) matmuls are the only
            # instructions the HAM counts as PE activity.  ~6us of them here
            # covers the first DMA waits and enters the kernel body warm.
            def emit_warm(n, N=128):
                warm_ps = ps_small.tile([128, 512], F32, tag="w0")
                for _ in range(n):
                    nc.tensor.matmul(
                        warm_ps[:, 0:N], identity[:], identity[:, 0:N],
                        start=True, stop=True,
                    )

            bias_bc = consts.tile([128, E], F32)

            def emit_bias_bcast():
                # broadcast bias to 128 partitions via a K=1 outer product
                bias_ps = ps_small.tile([128, E], F32, tag="w0")
                nc.tensor.matmul(
                    bias_ps[:], ones1[:], bias_sb[:], start=True, stop=True
                )
                nc.vector.tensor_copy(bias_bc[:], bias_ps[:])

            def emit_loads(c):
                co = c * S
                # load raw chunk as [128, 16, 64]: row p holds s = 128t+p
                q_raw3 = raw_pool.tile([128, KT, D], MM_DT, tag=f"q_raw{c}")
                k_raw3 = raw_pool.tile([128, KT, D], MM_DT, tag=f"k_raw{c}")
                v_raw3 = raw_pool.tile([128, KT, D], MM_DT, tag=f"v_raw{c}")
                for dst3, srcd in ((k_raw3, k_in), (q_raw3, q_in), (v_raw3, v_in)):
                    for hl in range(2):
                        nc.sync.dma_start(
                            dst3[:, 8 * hl : 8 * (hl + 1), :],
                            srcd[
                                co + 1024 * hl : co + 1024 * (hl + 1), :
                            ].rearrange("(t p) d -> p t d", p=128),
                        )
                return q_raw3, k_raw3, v_raw3

            def emit_prep(c, raws, act_copies=False):
                # act_copies: route the non-critical PSUM->SBUF evacuations
                # through the Scalar engine (idle during chunk-0 prep) so the
                # DVE queue stays on the khT2->khp->khpT2->scores chain.
                cp_aux = nc.scalar.copy if act_copies else nc.vector.tensor_copy
                q_raw3, k_raw3, v_raw3 = raws
                k_raw = k_raw3[:].rearrange("p t d -> p (t d)")
                q_raw = q_raw3[:].rearrange("p t d -> p (t d)")
                v_raw = v_raw3[:].rearrange("p t d -> p (t d)")

                # PE-transpose into split-partition layouts.  One [128,128]
                # fp16 transpose covers s-tiles t=2g (psum rows 0-63) and
                # t=2g+1 (rows 64-127).
                def transpose_psum(rawt):
                    ps_t = ps_small.tile([128, 1024], MM_DT, tag="w0")
                    for g in range(8):
                        nc.tensor.transpose(
                            ps_t[:, 128 * g : 128 * (g + 1)],
                            rawt[:, 128 * g : 128 * (g + 1)],
                            identity[:],
                        )
                    return ps_t[:].rearrange("x (g j) -> x g j", g=8)

                # khT2 [128, 8, 128]: [0:64, g] = kc^T s-tile 2g,
                #                     [64:128, g] = s-tile 2g+1 (no shift!)
                khT2 = tp_pool.tile([128, 8, 128], MM_DT, tag="khT")
                pv = transpose_psum(k_raw)
                nc.vector.tensor_copy(khT2[0:D, :, :], pv[0:D])
                nc.vector.tensor_copy(khT2[D:128, :, :], pv[D : 2 * D])

                # qhT2 [128, 16, 128]: both partition halves hold qc^T
                # (row tile T8 streams its rhs from partitions 64-127).
                qhT2 = tp_pool.tile([128, KT, 128], MM_DT, tag="qhT")
                qv = qhT2[:].rearrange("p (g h) j -> p h g j", h=2)
                pv = transpose_psum(q_raw)
                cp_aux(qv[0:D, 0], pv[0:D])
                cp_aux(qv[0:D, 1], pv[D : 2 * D])
                qrest = None
                if act_copies:
                    # only t<=7 (pair 0) duplicates to the hi partitions now;
                    # t 8-15 dups later as one cheap SBUF->SBUF DVE copy so
                    # the first EXP isn't queued behind them on ScalarE
                    cp_aux(qv[D:128, 0, 0:4], pv[0:D, 0:4])
                    cp_aux(qv[D:128, 1, 0:4], pv[D : 2 * D, 0:4])

                    def qrest():
                        nc.vector.tensor_copy(
                            qhT2[D:128, 8:KT, :], qhT2[0:D, 8:KT, :]
                        )
                else:
                    cp_aux(qv[D:128, 0], pv[0:D])
                    cp_aux(qv[D:128, 1], pv[D : 2 * D])

                # vhT [64, 2048] as in v1 (lhsT source for the V' projection)
                vhT = tp_pool.tile([D, S], MM_DT, tag="vhT")
                dv = vhT[:].rearrange("d (g h p) -> d h g p", g=8, h=2)
                pv = transpose_psum(v_raw)
                cp_aux(dv[:, 0], pv[0:D])
                cp_aux(dv[:, 1], pv[D : 2 * D])

                # khp^T = g_t.T @ khT, row-tiled 64x64: T0 handles even
                # s-tiles from SBUF partitions 0-63, T8 odd s-tiles from
                # 64-127, concurrently.  T8 writes PSUM partitions 0-63, so
                # both outputs are base-0 [64,512] tiles (ring -> different
                # banks as row tiling requires).
                # khpT2 [128, 8, 128]: [0:64, g] = khp^T for kt=2g,
                #                      [64:128, g] = kt=2g+1.
                khpT2 = tp_pool.tile([128, 8, 128], MM_DT, tag="khpT")
                khv = khT2[:].rearrange("p g j -> p (g j)")
                for n in range(2):
                    ps_e = ps_small.tile([D, 512], F32, tag="w0")
                    ps_o = ps_small.tile([D, 512], F32, tag="w0")
                    nc.tensor.matmul(
                        ps_e[:],
                        g2[0:D, :],
                        khv[0:D, 512 * n : 512 * (n + 1)],
                        start=True, stop=True,
                    )
                    nc.tensor.matmul(
                        ps_o[:],
                        g2[D:128, :],
                        khv[D:128, 512 * n : 512 * (n + 1)],
                        start=True, stop=True,
                    )
                    nc.vector.tensor_copy(
                        khpT2[0:D, 4 * n : 4 * (n + 1), :],
                        ps_e[:].rearrange("p (g j) -> p g j", g=4),
                    )
                    nc.vector.tensor_copy(
                        khpT2[D:128, 4 * n : 4 * (n + 1), :],
                        ps_o[:].rearrange("p (g j) -> p g j", g=4),
                    )

                # V' = vc @ Wv.T with ones column LAST: vp3 [128, 16, 65],
                # column 64 = ones (softmax sums land at pav partition 64,
                # which is 32-aligned so a DVE copy can extract it).
                vp = raw_pool.tile([128, KT * (D + 1)], MM_DT, tag="vp")
                vp3 = vp[:].rearrange("p (kt x) -> p kt x", x=D + 1)
                nc.vector.tensor_copy(vp3[:, :, D : D + 1], ones_col[:])
                for half in range(2):  # 8 projections of N=64 per psum bank
                    ps_v = ps_small.tile([128, 512], F32, tag="w0")
                    for m in range(8):
                        kt = 8 * half + m
                        nc.tensor.matmul(
                            ps_v[:, D * m : D * (m + 1)],
                            vhT[:, 128 * kt : 128 * (kt + 1)],
                            wv_sb[:],
                            start=True, stop=True,
                        )
                    cp_aux(
                        vp3[:, 8 * half : 8 * half + 8, 0:D],
                        ps_v[:].rearrange("p (m x) -> p m x", x=D),
                    )
                return (qhT2, khpT2, vp3), qrest

            def attn_state(c):
                atT = at_pool.tile([D, S], MM_DT, tag=f"at{c}")
                return atT, {}, {}

            def emit_attention_pair(c, qhT2, khpT2, vp3, st8, pair, inject=None):
                # two interleaved q-block chains (A, B) per pair phase; a
                # score tile holds TWO kt tiles (written by a concurrent
                # row-tiled MM pair) -> one exp per [128,1024]
                atT, pcps, rss = st8
                qoA = 1024 * pair
                qoB = qoA + 512
                pavA = ps_acc.tile([D + 1, 512], F32, tag="acc")
                pavB = ps_acc.tile([D + 1, 512], F32, tag="acc")
                qview = qhT2[:].rearrange("p t j -> p (t j)")
                for g in range(KT // 2):
                    if g == 3 and inject is not None:
                        inject()
                    pts = []
                    for qo in (qoA, qoB):
                        st = ps_score.tile([128, 1024], F32, tag="score")
                        # concurrent row-tiled pair: T0 computes kt=2g from
                        # partitions 0-63, T8 computes kt=2g+1 from 64-127.
                        nc.tensor.matmul(
                            st[:, 0:512],
                            khpT2[0:D, g, :],
                            qview[0:D, qo : qo + 512],
                            start=True, stop=True,
                        )
                        nc.tensor.matmul(
                            st[:, 512:1024],
                            khpT2[D:128, g, :],
                            qview[D:128, qo : qo + 512],
                            start=True, stop=True,
                        )
                        ptile = pt_pool.tile([128, 1024], MM_DT, tag="pt")
                        nc.scalar.activation(ptile[:], st[:], ACT_EXP, scale=0.125)
                        pts.append(ptile)
                    for u in range(2):
                        kt = 2 * g + u
                        for pav, ptile in zip((pavA, pavB), pts):
                            nc.tensor.matmul(
                                pav[:],
                                vp3[:, kt],
                                ptile[:, 512 * u : 512 * (u + 1)],
                                start=(kt == 0),
                                stop=(kt == KT - 1),
                            )
                # free both accumulation banks right away.  The sums row
                # (partition 64, 32-aligned) copies down to a base-0 [1,512]
                # tile for the custom-DVE reciprocal; 1/s then casts to fp16
                # so the broadcast matmul streams at full fp16 rate.
                for pav, qb in ((pavA, 2 * pair), (pavB, 2 * pair + 1)):
                    release_pav(st8, pav, qb)

            def release_pav(st8, pav, qb):
                atT, pcps, rss = st8
                s_sb = npool.tile([1, 512], F32, tag=f"s{qb}")
                nc.vector.tensor_copy(s_sb[:], pav[D : D + 1, :])
                rs = npool.tile([1, 512], F32, tag=f"rsf{qb}")
                nc.vector.reciprocal_approx_fast(rs[:], s_sb[:])
                rs16 = npool.tile([1, 512], MM_DT, tag=f"rs{qb}")
                nc.vector.tensor_copy(rs16[:], rs[:])
                rss[qb] = rs16
                acp = npool.tile([D, 512], F32, tag=f"acp{qb}")
                nc.vector.tensor_copy(acp[:], pav[0:D, :])
                pcps[qb] = acp

            def emit_attention_pair_serial(
                c, qhT2, khpT2, vp3, st8, pair, inject=None
            ):
                # LAST pair phase: run chain A (qoA) to completion before
                # chain B, so A's normalization chain overlaps B's compute
                # and only B's norm + FC + DMA remain serial at the end.
                atT, pcps, rss = st8
                qview = qhT2[:].rearrange("p t j -> p (t j)")
                for chain in range(2):
                    qo = 1024 * pair + 512 * chain
                    qb = 2 * pair + chain
                    pav = ps_acc.tile([D + 1, 512], F32, tag="acc")
                    for g in range(KT // 2):
                        if chain == 0 and g == 3 and inject is not None:
                            inject()
                        st = ps_score.tile([128, 1024], F32, tag="score")
                        nc.tensor.matmul(
                            st[:, 0:512],
                            khpT2[0:D, g, :],
                            qview[0:D, qo : qo + 512],
                            start=True, stop=True,
                        )
                        nc.tensor.matmul(
                            st[:, 512:1024],
                            khpT2[D:128, g, :],
                            qview[D:128, qo : qo + 512],
                            start=True, stop=True,
                        )
                        ptile = pt_pool.tile([128, 1024], MM_DT, tag="pt")
                        nc.scalar.activation(ptile[:], st[:], ACT_EXP, scale=0.125)
                        for u in range(2):
                            kt = 2 * g + u
                            nc.tensor.matmul(
                                pav[:],
                                vp3[:, kt],
                                ptile[:, 512 * u : 512 * (u + 1)],
                                start=(kt == 0),
                                stop=(kt == KT - 1),
                            )
                    release_pav(st8, pav, qb)
                    if chain == 0:
                        norm_qb(st8, qb)

            def norm_qb(st8, qb):
                atT, pcps, rss = st8
                rb_ps = ps_small.tile([D, 512], F32, tag="w0")
                nc.tensor.matmul(
                    rb_ps[:], ones_row[:], rss[qb][:], start=True, stop=True
                )
                nc.vector.tensor_mul(
                    atT[:, 512 * qb : 512 * (qb + 1)],
                    pcps[qb][:],
                    rb_ps[:],
                )

            def norm_pe(st8, pair):
                # normalize the pair's two q-blocks: PE ones-broadcast of
                # 1/s (K=1 outer product) then a DVE multiply; injected into
                # the NEXT phase's stream where rs is long since ready
                for qb in (2 * pair, 2 * pair + 1):
                    norm_qb(st8, qb)

            def emit_tail(c, st8, halves=(0, 1)):
                atT, pcps, rss = st8
                atv = atT[:].rearrange("d (m r j) -> d m j r", m=2, j=8)

                # FC: out rows rr (128 per r-tile), 8 accumulating matmuls
                for half in halves:
                    po = ps_small.tile([128, E], F32, tag="w0")
                    for j in range(8):
                        nc.tensor.matmul(
                            po[:],
                            atv[:, half, j, :],
                            wfc_sb[:, j, :],
                            start=(j == 0),
                            stop=(j == 7),
                        )
                    ot = out_pool.tile([128, E], F32, tag="out")
                    nc.vector.tensor_add(ot[:], po[:], bias_bc[:])
                    nc.sync.dma_start(
                        out[256 * c + 128 * half : 256 * c + 128 * (half + 1), :],
                        ot[:],
                    )

            # software-pipeline the chunks so the PE queue always has
            # dependency-free work at every phase seam: chunk1's prep fills
            # the attn0->attn1 seam; chunk0's tail fills the seam between
            # chunk1's two pair-phases.
            emit_consts_dmas()
            emit_warm(18)
            emit_bias_bcast()
            raw0 = emit_loads(0)
            t0, qrest0 = emit_prep(0, raw0, act_copies=True)
            s0 = attn_state(0)
            emit_attention_pair(0, *t0, s0, 0, inject=qrest0)
            emit_attention_pair(0, *t0, s0, 1, inject=lambda: norm_pe(s0, 0))
            raw1 = emit_loads(1)
            t1, _ = emit_prep(1, raw1)
            s1 = attn_state(1)
            emit_attention_pair(1, *t1, s1, 0, inject=lambda: norm_pe(s0, 1))
            emit_tail(0, s0)

            def inject_last():
                norm_pe(s1, 0)
                emit_tail(1, s1, halves=(0,))

            emit_attention_pair_serial(1, *t1, s1, 1, inject=inject_last)
            norm_qb(s1, 3)
            emit_tail(1, s1, halves=(1,))

    nc.compile()
    return nc


_NC_CACHE = None


def _get_nc():
    global _NC_CACHE
    if _NC_CACHE is None:
        _NC_CACHE = build_core_program()
    return _NC_CACHE


def make_in_maps(q, k, v, Wq, Wk, Wv, Wfc, bfc):
    f16 = np.float16
    q = np.ascontiguousarray(q, dtype=np.float32)
    k = np.ascontiguousarray(k, dtype=np.float32)
    v = np.ascontiguousarray(v, dtype=np.float32)
    g_t = (
        (np.asarray(Wk, np.float32).T @ np.asarray(Wq, np.float32))
        .astype(f16)
    )
    wv_t = np.ascontiguousarray(np.asarray(Wv, np.float32).T.astype(f16))
    wfc_t = np.ascontiguousarray(np.asarray(Wfc, np.float32).T.astype(f16))
    bias = np.asarray(bfc, np.float32).reshape(1, E)

    qf = q.reshape(-1).astype(f16)
    kf = k.reshape(-1).astype(f16)
    vf = v.reshape(-1).astype(f16)
    C = S * D
    in_maps = []
    for i in range(NCORES):
        lo = 2 * i * C
        hi = (2 * i + 2) * C
        in_maps.append(
            dict(
                q_in=np.ascontiguousarray(qf[lo:hi].reshape(2 * S, D)),
                k_in=np.ascontiguousarray(kf[lo:hi].reshape(2 * S, D)),
                v_in=np.ascontiguousarray(vf[lo:hi].reshape(2 * S, D)),
                g_t=g_t,
                wv_t=wv_t,
                wfc_t=wfc_t,
                bias=bias,
            )
        )
    return in_maps


def kernel(q, k, v, Wq, Wk, Wv, Wfc, bfc, _trace=False):
    nc = _get_nc()
    in_maps = make_in_maps(q, k, v, Wq, Wk, Wv, Wfc, bfc)
    res = bass_utils.run_bass_kernel_spmd(
        nc, in_maps, core_ids=list(range(NCORES)), trace=_trace
    )
    out = np.concatenate([res.results[i]["out"] for i in range(NCORES)], axis=0)
    kernel.last_exec_time_ns = res.exec_time_ns
    kernel.last_results = res
    return out.reshape(S, 2, E)
